# revision 23
# baseline (speedup 1.0000x reference)
"""Trainium2 Bass kernel for nn_Attention_47725676593424.

GQA attention layer: B=2, S=2048, D=1024, H=16 q-heads, KVH=4 kv-heads,
HD=64, RoPE, causal mask, returns (out, new_k, new_v).

Sharding (8 cores): core = b*4 + g, b = batch (data parallel), g = head
group (tensor parallel). Each core computes q-heads [4g, 4g+4) and kv
head g for batch b (whole KV group local, GQA repeat is implicit), then
AllGathers the per-head attention output within its 4-core batch group
and computes a 256-column slice of the o_proj output.

Layout: activations flow transposed (feature on partitions, sequence on
the free axis) so every matmul contracts on the partition dim with zero
on-device transposes of x. Scores are computed transposed S^T[k, q]; the
softmax denominator comes free from a ones-column appended to V. The
causal mask is applied structurally (upper-triangular key blocks are
skipped / zeroed), which matches the reference's additive -1e9 mask
exactly because exp underflows to 0. Softmax runs without max
subtraction: scores*scale is bounded (|s| < ~4) for any plausible
activation scale here, so exp cannot overflow.
"""

import os
import numpy as np
from contextlib import ExitStack

import concourse.bass as bass
import concourse.mybir as mybir
import concourse.tile as tile
from concourse import bacc
from concourse.bass_utils import run_bass_kernel_spmd
from concourse.masks import make_identity

# Problem constants (hardcoded per harness contract).
B, S, D = 2, 2048, 1024
H, KVH, HD = 16, 4, 64
NCORES = 8
G = 4                 # head groups (tensor-parallel degree per batch)
HPG = H // G          # 4 q heads per core
DQ = HPG * HD         # 256 = per-core q/attn feature dim
SCALE = 1.0 / 8.0     # 1/sqrt(HD)
PAN = 512             # q panel width (one PSUM bank of fp32)
NPAN = S // PAN       # 4
SB = 128              # s block
NSB = S // SB         # 16
FCH = D // 128        # 8 feature chunks of the contraction dim
DT = mybir.dt.float32
BF = mybir.dt.bfloat16
F32 = np.float32
try:
    import ml_dtypes
    NPBF = ml_dtypes.bfloat16
except ImportError:  # pragma: no cover
    NPBF = None

_NC_CACHE = {}


def _build_nc():
    nc = bacc.Bacc("TRN2", target_bir_lowering=False, debug=False,
                   num_devices=NCORES)

    xT_h = nc.dram_tensor("xT", [D, S], BF, kind="ExternalInput")
    c64_h = nc.dram_tensor("c64", [HD, S], DT, kind="ExternalInput")
    s64_h = nc.dram_tensor("s64pm", [HD, S], DT, kind="ExternalInput")
    wq_h = nc.dram_tensor("wq", [D, DQ], BF, kind="ExternalInput")
    wk_h = nc.dram_tensor("wk", [D, HD], BF, kind="ExternalInput")
    wv_h = nc.dram_tensor("wv", [D, HD], BF, kind="ExternalInput")
    wo_h = nc.dram_tensor("wo", [D, DQ], BF, kind="ExternalInput")
    out_h = nc.dram_tensor("out_s", [S, DQ], DT, kind="ExternalOutput")
    kout_h = nc.dram_tensor("k_out", [S, HD], DT, kind="ExternalOutput")
    vout_h = nc.dram_tensor("v_out", [S, HD], DT, kind="ExternalOutput")

    xT, c64, s64pm = xT_h.ap(), c64_h.ap(), s64_h.ap()
    wq, wk, wv, wo = wq_h.ap(), wk_h.ap(), wv_h.ap(), wo_h.ap()
    out_s, k_out, v_out = out_h.ap(), kout_h.ap(), vout_h.ap()

    with ExitStack() as ctx:
        tc = ctx.enter_context(tile.TileContext(nc))
        _emit(ctx, tc, nc, xT, c64, s64pm, wq, wk, wv, wo,
              out_s, k_out, v_out)

    nc.compile()
    return nc


def _emit(ctx, tc, nc, xT, c64, s64pm, wq, wk, wv, wo,
          out_s, k_out, v_out):
    EXP = mybir.ActivationFunctionType.Exp

    consts = ctx.enter_context(tc.tile_pool(name="consts", bufs=1))
    big = ctx.enter_context(tc.tile_pool(name="big", bufs=8))
    qkv = ctx.enter_context(tc.tile_pool(name="qkv", bufs=1))
    pt_pool = ctx.enter_context(tc.tile_pool(name="pt", bufs=3))
    tmp = ctx.enter_context(tc.tile_pool(name="tmp", bufs=6))
    outp = ctx.enter_context(tc.tile_pool(name="outp", bufs=3))
    dram = ctx.enter_context(tc.tile_pool(name="dram", bufs=1, space="DRAM"))

    # ---- inputs into SBUF (x and projection weights first: they gate the
    # first matmuls; rope tables and wo aren't needed until later) --------
    xt_sb = []
    for c in range(FCH):
        t = big.tile([128, S], BF, name=f"xt{c}", tag="big")
        nc.sync.dma_start(out=t, in_=xT[c * 128:(c + 1) * 128, :])
        xt_sb.append(t)

    wq_sb = consts.tile([128, FCH, DQ], BF)
    wk_sb = consts.tile([128, FCH, HD], BF)
    wv_sb = consts.tile([128, FCH, HD], BF)
    wo_sb = consts.tile([128, FCH, DQ], BF)
    for dst, src_ap in ((wq_sb, wq), (wk_sb, wk), (wv_sb, wv)):
        nc.sync.dma_start(out=dst,
                          in_=src_ap.rearrange("(c p) d -> p c d", p=128))

    # c128/s128pm: [cos;cos] and [-sin;+sin] replicated to all 4 head rows
    c128 = consts.tile([128, S], DT)
    s128 = consts.tile([128, S], DT)
    nc.sync.dma_start(out=c128[0:HD, :], in_=c64)
    nc.sync.dma_start(out=c128[HD:128, :], in_=c64)
    nc.sync.dma_start(out=s128[0:HD, :], in_=s64pm)
    nc.sync.dma_start(out=s128[HD:128, :], in_=s64pm)
    nc.sync.dma_start(out=wo_sb,
                      in_=wo.rearrange("(c p) d -> p c d", p=128))

    ident = consts.tile([128, 128], DT)
    make_identity(nc, ident)
    # bf16 identity + additive -1e9 upper-triangle: the causal mask is
    # accumulated into the diagonal score blocks by one extra PE matmul
    # (ident^T @ mneg = mneg), keeping the kb chain PE->ACT->PE only.
    ident_bf = consts.tile([128, 128], BF)
    nc.gpsimd.memset(ident_bf, 0.0)
    nc.gpsimd.affine_select(
        out=ident_bf, in_=ident_bf, compare_op=mybir.AluOpType.not_equal,
        fill=1.0, base=0, pattern=[[-1, 128]], channel_multiplier=1)
    mneg = consts.tile([128, 128], BF)
    nc.gpsimd.memset(mneg, 0.0)
    nc.gpsimd.affine_select(
        out=mneg, in_=mneg, compare_op=mybir.AluOpType.is_ge,
        fill=-1e9, base=0, pattern=[[1, 128]], channel_multiplier=-1)

    # Persistent transposed activations.
    qT_sb = qkv.tile([128, 2, S], BF)       # 2 packs x (2 heads x 64)
    # k master in fp32 (feeds the k_out output); bf16 copy duplicated in
    # both partition halves so scores lhsT can match the base partition
    # (0 or 64) of each q head's rhs slice.
    kT_f32 = qkv.tile([HD, S], DT)
    kT_bf = qkv.tile([128, S], BF)
    vT_sb = qkv.tile([HD, S], DT)           # pre-transpose v (fp32 master)
    v_ext = qkv.tile([128, NSB, 65], DT)    # v natural + ones column
    v_ext_bf = qkv.tile([128, NSB, 65], BF)
    att0 = qkv.tile([128, S], BF)           # attn out^T, heads 0,1
    att1 = qkv.tile([128, S], BF)           # attn out^T, heads 2,3

    # ---- phase 1: QKV projections + RoPE (own PSUM pool scope) ------------
    # RoPE via a second "swapped" projection computed on the PE:
    # rot(q) = q * [cos;cos] + q_swap * [-sin;+sin], all full-width DVE ops.
    with tc.tile_pool(name="psA", bufs=4, space="PSUM") as psA:
        for sp in range(NPAN):
            for pk in range(2):
                sl = slice(sp * PAN, (sp + 1) * PAN)
                q_ps = psA.tile([128, PAN], DT, name="q_ps", tag="ps")
                for c in range(FCH):
                    nc.tensor.matmul(
                        q_ps,
                        wq_sb[:, c, pk * 128:(pk + 1) * 128],
                        xt_sb[c][:, sl],
                        start=(c == 0), stop=(c == FCH - 1))
                t1 = tmp.tile([128, PAN], DT, name="rt1", tag="ropet")
                t2 = tmp.tile([128, PAN], DT, name="rt2", tag="ropet")
                nc.vector.tensor_mul(t1, q_ps, c128[:, sl])
                # swapped-half reads straight from PSUM (mixed PSUM+SBUF
                # operands may have different base partitions)
                for q in range(4):
                    lo_d, hi_d = q * 32, q * 32 + 32
                    sw = (q ^ 1) * 32
                    nc.vector.tensor_mul(t2[lo_d:hi_d, :],
                                         q_ps[sw:sw + 32, :],
                                         s128[lo_d:hi_d, sl])
                nc.vector.tensor_add(qT_sb[:, pk, sl], t1, t2)

        for sp in range(NPAN):
            sl = slice(sp * PAN, (sp + 1) * PAN)
            k_ps = psA.tile([HD, PAN], DT, name="k_ps", tag="ps")
            for c in range(FCH):
                nc.tensor.matmul(
                    k_ps, wk_sb[:, c, :], xt_sb[c][:, sl],
                    start=(c == 0), stop=(c == FCH - 1))
            t1k = tmp.tile([HD, PAN], DT, name="rt1k", tag="ropet")
            t2k = tmp.tile([HD, PAN], DT, name="rt2k", tag="ropet")
            nc.vector.tensor_mul(t1k, k_ps, c128[0:HD, sl])
            for q in range(2):
                lo_d, hi_d = q * 32, q * 32 + 32
                sw = (q ^ 1) * 32
                nc.vector.tensor_mul(t2k[lo_d:hi_d, :], k_ps[sw:sw + 32, :],
                                     s128[lo_d:hi_d, sl])
            nc.vector.tensor_add(kT_f32[:, sl], t1k, t2k)
            nc.vector.tensor_copy(kT_bf[0:HD, sl], kT_f32[:, sl])
            nc.sync.dma_start(out=kT_bf[HD:128, sl], in_=kT_bf[0:HD, sl])

        for sp in range(NPAN):
            v_ps = psA.tile([HD, PAN], DT, name="v_ps", tag="ps")
            for c in range(FCH):
                nc.tensor.matmul(
                    v_ps, wv_sb[:, c, :],
                    xt_sb[c][:, sp * PAN:(sp + 1) * PAN],
                    start=(c == 0), stop=(c == FCH - 1))
            nc.scalar.copy(vT_sb[:, sp * PAN:(sp + 1) * PAN], v_ps)

        # ---- phase 2: k/v back to natural layout for outputs + AV --------
        nc.vector.memset(v_ext[:, :, 64:65], 1.0)
        for kb in range(NSB):
            sl = slice(kb * 128, (kb + 1) * 128)
            vt_ps = psA.tile([128, HD], DT, name="vt_ps", tag="tp")
            nc.tensor.transpose(vt_ps, vT_sb[:, sl], ident[0:HD, 0:HD])
            nc.vector.tensor_copy(v_ext[:, kb, 0:HD], vt_ps)
            nc.sync.dma_start(out=v_out[sl, :], in_=v_ext[:, kb, 0:HD])

            kt_ps = psA.tile([128, HD], DT, name="kt_ps", tag="tp")
            nc.tensor.transpose(kt_ps, kT_f32[:, sl], ident[0:HD, 0:HD])
            kn_sb = outp.tile([128, HD], DT, name="kn_sb", tag="kn")
            nc.vector.tensor_copy(kn_sb, kt_ps)
            nc.sync.dma_start(out=k_out[sl, :], in_=kn_sb)
            nc.vector.tensor_copy(v_ext_bf[:, kb, :], v_ext[:, kb, :])

    # ---- phase 3: causal attention, scores transposed --------------------
    # Panel-outer: both head-pairs of a 512-column panel finish together,
    # so the attn^T AllGather + o_proj for the first S/2 columns overlap
    # the attention compute of the second S/2.
    cc_ins = [dram.tile([2, 128, PAN], BF, name=f"cc_in{i}")
              for i in range(NPAN)]
    cc_outs = [dram.tile([G, 2, 128, PAN], BF, name=f"cc_out{i}")
               for i in range(NPAN)]
    at_sb = [[None] * FCH for _ in range(NPAN)]

    with tc.tile_pool(name="psS", bufs=2, space="PSUM") as psS, \
         tc.tile_pool(name="psAV", bufs=1, space="PSUM") as psAV, \
         tc.tile_pool(name="psO", bufs=2, space="PSUM") as psO:

        def o_proj_quarter(p):
            # AllGather panel p of attn^T, then its 4 o_proj s-blocks.
            for att, i in ((att0, 0), (att1, 1)):
                nc.sync.dma_start(out=cc_ins[p][i],
                                  in_=att[:, p * PAN:(p + 1) * PAN])
            nc.gpsimd.collective_compute(
                "AllGather", mybir.AluOpType.bypass,
                replica_groups=[[0, 1, 2, 3], [4, 5, 6, 7]],
                ins=[cc_ins[p].opt()], outs=[cc_outs[p].opt()])
            # cc_outs[p][r][i] holds heads {4r+2i, 4r+2i+1} = wo chunk
            # 2r+i for columns [p*PAN, (p+1)*PAN).
            for i in range(2):
                for r in range(G):
                    t = big.tile([128, PAN], BF, name=f"at{p}_{i}_{r}",
                                 tag="at")
                    nc.sync.dma_start(out=t, in_=cc_outs[p][r, i])
                    at_sb[p][2 * r + i] = t
            for sb_i in range(4):
                sl = slice(p * PAN + sb_i * 128, p * PAN + (sb_i + 1) * 128)
                o_ps = psO.tile([128, DQ], DT, name="o_ps", tag="o")
                for c in range(FCH):
                    nc.tensor.matmul(
                        o_ps, at_sb[p][c][:, sb_i * 128:(sb_i + 1) * 128],
                        wo_sb[:, c, :], start=(c == 0), stop=(c == FCH - 1))
                o_sb = outp.tile([128, DQ], DT, name="o_sb", tag="o_sb")
                nc.vector.tensor_copy(o_sb, o_ps)
                nc.sync.dma_start(out=out_s[sl, :], in_=o_sb)

        for p in range(NPAN):
            nkb = 4 * (p + 1)
            q_sl = slice(p * PAN, (p + 1) * PAN)
            for hp in range(2):
                att = att0 if hp == 0 else att1
                av_ps = psAV.tile([65, 2, PAN], DT, name="av_ps", tag="av")
                for kb in range(nkb):
                    k_sl = slice(kb * 128, (kb + 1) * 128)
                    off = (kb - 4 * p) * 128  # >=0 only on diagonal blocks
                    lo = max(off, 0)  # first valid q column in this panel
                    s_ps = psS.tile([128, 2, PAN], DT, name="s_ps", tag="s")
                    diag = off >= 0
                    for hi in range(2):
                        base = hi * 64
                        nc.tensor.matmul(
                            s_ps[:, hi, lo:],
                            kT_bf[base:base + 64, k_sl],
                            qT_sb[base:base + 64, hp,
                                  p * PAN + lo:(p + 1) * PAN],
                            start=True, stop=not diag)
                    if diag:
                        for hi in range(2):
                            nc.tensor.matmul(
                                s_ps[:, hi, off:off + 128],
                                ident_bf, mneg, start=False, stop=True)
                    pt = pt_pool.tile([128, 2, PAN], BF, name="pt", tag="pt")
                    nc.scalar.activation(pt[:, :, lo:], s_ps[:, :, lo:],
                                         EXP, scale=SCALE)
                    for hi in range(2):
                        nc.tensor.matmul(
                            av_ps[:, hi, lo:], v_ext_bf[:, kb, :],
                            pt[:, hi, lo:],
                            start=(kb == 0), stop=(kb == nkb - 1))
                # Evacuate the av bank quickly (copy unnormalized), then
                # normalize in place once 1/Z arrives via the DRAM-bounce
                # partition broadcast (engines can't partition-broadcast;
                # gpsimd must stay free for the collectives).
                z_sb = tmp.tile([1, 2, PAN], DT, name="z_sb", tag="z")
                nc.vector.tensor_copy(z_sb, av_ps[64:65, :, :])
                for hi in range(2):
                    nc.vector.tensor_copy(att[hi * 64:hi * 64 + 64, q_sl],
                                          av_ps[0:HD, hi, :])
                r_sb = tmp.tile([1, 2, PAN], DT, name="r_sb", tag="r")
                nc.vector.reciprocal_approx_fast(out=r_sb, in_=z_sb)
                r_dr = dram.tile([1, 2, PAN], DT, name="r_dr", tag="r_dr",
                                 bufs=2)
                nc.sync.dma_start(out=r_dr, in_=r_sb)
                rb = tmp.tile([128, PAN], DT, name="rb", tag="rb")
                for hi in range(2):
                    nc.sync.dma_start(
                        out=rb[hi * 64:hi * 64 + 64, :],
                        in_=r_dr[0:1, hi, :].to_broadcast([HD, PAN]))
                for hi in range(2):
                    sl_a = slice(hi * 64, hi * 64 + 64)
                    nc.vector.tensor_mul(att[sl_a, q_sl], att[sl_a, q_sl],
                                         rb[sl_a, :])
            o_proj_quarter(p)


def get_nc():
    if "nc" not in _NC_CACHE:
        _NC_CACHE["nc"] = _build_nc()
    return _NC_CACHE["nc"]


def _swap_halves(w):
    """Swap the two 32-wide halves of every 64-wide head block of w's
    columns (so q_swap = x @ w_swap has rotate-half partner rows)."""
    w = w.reshape(w.shape[0], -1, 2, HD // 2)
    return np.ascontiguousarray(w[:, :, ::-1, :].reshape(w.shape[0], -1))


def make_in_maps(x, cos, sin, wq, wk, wv, wo):
    cosT = np.asarray(cos, F32).T
    sinT = np.asarray(sin, F32).T
    c64 = np.ascontiguousarray(np.vstack([cosT, cosT]))
    s64pm = np.ascontiguousarray(np.vstack([-sinT, sinT]))
    x = np.asarray(x, F32).astype(NPBF)
    wq, wk, wv, wo = (np.asarray(a, F32).astype(NPBF)
                      for a in (wq, wk, wv, wo))
    in_maps = []
    for core in range(NCORES):
        b, g = divmod(core, G)
        in_maps.append({
            "xT": np.ascontiguousarray(x[b].T),
            "c64": c64,
            "s64pm": s64pm,
            "wq": np.ascontiguousarray(wq[:, g * DQ:(g + 1) * DQ]),
            "wk": np.ascontiguousarray(wk[:, g * HD:(g + 1) * HD]),
            "wv": np.ascontiguousarray(wv[:, g * HD:(g + 1) * HD]),
            "wo": np.ascontiguousarray(wo[:, g * DQ:(g + 1) * DQ]),
        })
    return in_maps


def assemble(results):
    out = np.empty((B, S, D), F32)
    new_k = np.empty((B, S, KVH, HD), F32)
    new_v = np.empty((B, S, KVH, HD), F32)
    for core in range(NCORES):
        b, g = divmod(core, G)
        r = results[core]
        out[b, :, g * DQ:(g + 1) * DQ] = r["out_s"]
        new_k[b, :, g, :] = r["k_out"]
        new_v[b, :, g, :] = r["v_out"]
    return out, new_k, new_v


def _ensure_ntff_hook():
    """Register the axon NTFF profile hook if the container's antenv stub
    lacks it (needed only for trace=True timing runs)."""
    import sys
    import types
    try:
        from antenv.axon_hooks import get_axon_ntff_profile_hook  # noqa: F401
        return
    except ImportError:
        pass
    try:
        import antenv
        from trn_agent_boot.trn_boot import _ntff_profile_via_ctypes
        mod = types.ModuleType("antenv.axon_hooks")
        state = {"fn": None}
        mod.set_axon_ntff_profile_hook = lambda fn: state.update(fn=fn)
        mod.get_axon_ntff_profile_hook = lambda: state["fn"]
        sys.modules["antenv.axon_hooks"] = mod
        antenv.axon_hooks = mod
        hook = _ntff_profile_via_ctypes("/opt/axon/libaxon_pjrt.so")
        if hook is not None:
            mod.set_axon_ntff_profile_hook(hook)
    except Exception as e:  # profiling is best-effort; never break the run
        print(f"ntff hook setup failed: {e}")


def kernel(x, cos, sin, mask, wq, wk, wv, wo):
    # mask is not shipped to the device: the kernel applies causality
    # structurally, which matches the reference's -1e9 additive mask.
    nc = get_nc()
    in_maps = make_in_maps(x, cos, sin, wq, wk, wv, wo)
    trace = bool(int(os.environ.get("KERNEL_TRACE", "0")))
    if trace:
        _ensure_ntff_hook()
    res = run_bass_kernel_spmd(nc, in_maps, list(range(NCORES)), trace=trace)
    if trace:
        _NC_CACHE["last_exec_time_ns"] = res.exec_time_ns
    return assemble(res.results)


# revision 24
# speedup vs baseline: 1.0100x; 1.0100x over previous
"""Trainium2 Bass kernel for nn_Attention_47725676593424.

GQA attention layer: B=2, S=2048, D=1024, H=16 q-heads, KVH=4 kv-heads,
HD=64, RoPE, causal mask, returns (out, new_k, new_v).

Sharding (8 cores): core = b*4 + g, b = batch (data parallel), g = head
group (tensor parallel). Each core computes q-heads [4g, 4g+4) and kv
head g for batch b (whole KV group local, GQA repeat is implicit), then
AllGathers the per-head attention output within its 4-core batch group
and computes a 256-column slice of the o_proj output.

Layout: activations flow transposed (feature on partitions, sequence on
the free axis) so every matmul contracts on the partition dim with zero
on-device transposes of x. Scores are computed transposed S^T[k, q]; the
softmax denominator comes free from a ones-column appended to V. The
causal mask is applied structurally (upper-triangular key blocks are
skipped / zeroed), which matches the reference's additive -1e9 mask
exactly because exp underflows to 0. Softmax runs without max
subtraction: scores*scale is bounded (|s| < ~4) for any plausible
activation scale here, so exp cannot overflow.
"""

import os
import numpy as np
from contextlib import ExitStack

import concourse.bass as bass
import concourse.mybir as mybir
import concourse.tile as tile
from concourse import bacc
from concourse.bass_utils import run_bass_kernel_spmd
from concourse.masks import make_identity

# Problem constants (hardcoded per harness contract).
B, S, D = 2, 2048, 1024
H, KVH, HD = 16, 4, 64
NCORES = 8
G = 4                 # head groups (tensor-parallel degree per batch)
HPG = H // G          # 4 q heads per core
DQ = HPG * HD         # 256 = per-core q/attn feature dim
SCALE = 1.0 / 8.0     # 1/sqrt(HD)
PAN = 512             # q panel width (one PSUM bank of fp32)
NPAN = S // PAN       # 4
SB = 128              # s block
NSB = S // SB         # 16
FCH = D // 128        # 8 feature chunks of the contraction dim
DT = mybir.dt.float32
BF = mybir.dt.bfloat16
F32 = np.float32
try:
    import ml_dtypes
    NPBF = ml_dtypes.bfloat16
except ImportError:  # pragma: no cover
    NPBF = None

_NC_CACHE = {}


def _build_nc():
    nc = bacc.Bacc("TRN2", target_bir_lowering=False, debug=False,
                   num_devices=NCORES)

    xT_h = nc.dram_tensor("xT", [D, S], BF, kind="ExternalInput")
    c64_h = nc.dram_tensor("c64", [HD, S], DT, kind="ExternalInput")
    s64_h = nc.dram_tensor("s64pm", [HD, S], DT, kind="ExternalInput")
    wq_h = nc.dram_tensor("wq", [D, DQ], BF, kind="ExternalInput")
    wk_h = nc.dram_tensor("wk", [D, HD], BF, kind="ExternalInput")
    wv_h = nc.dram_tensor("wv", [D, HD], BF, kind="ExternalInput")
    wo_h = nc.dram_tensor("wo", [D, DQ], BF, kind="ExternalInput")
    out_h = nc.dram_tensor("out_s", [S, DQ], DT, kind="ExternalOutput")
    kout_h = nc.dram_tensor("k_out", [S, HD], DT, kind="ExternalOutput")
    vout_h = nc.dram_tensor("v_out", [S, HD], DT, kind="ExternalOutput")

    xT, c64, s64pm = xT_h.ap(), c64_h.ap(), s64_h.ap()
    wq, wk, wv, wo = wq_h.ap(), wk_h.ap(), wv_h.ap(), wo_h.ap()
    out_s, k_out, v_out = out_h.ap(), kout_h.ap(), vout_h.ap()

    with ExitStack() as ctx:
        tc = ctx.enter_context(tile.TileContext(nc))
        _emit(ctx, tc, nc, xT, c64, s64pm, wq, wk, wv, wo,
              out_s, k_out, v_out)

    nc.compile()
    return nc


def _emit(ctx, tc, nc, xT, c64, s64pm, wq, wk, wv, wo,
          out_s, k_out, v_out):
    EXP = mybir.ActivationFunctionType.Exp

    consts = ctx.enter_context(tc.tile_pool(name="consts", bufs=1))
    big = ctx.enter_context(tc.tile_pool(name="big", bufs=8))
    qkv = ctx.enter_context(tc.tile_pool(name="qkv", bufs=1))
    pt_pool = ctx.enter_context(tc.tile_pool(name="pt", bufs=3))
    tmp = ctx.enter_context(tc.tile_pool(name="tmp", bufs=6))
    outp = ctx.enter_context(tc.tile_pool(name="outp", bufs=3))
    dram = ctx.enter_context(tc.tile_pool(name="dram", bufs=1, space="DRAM"))

    # ---- inputs into SBUF (x and projection weights first: they gate the
    # first matmuls; rope tables and wo aren't needed until later) --------
    wq_sb = consts.tile([128, FCH, DQ], BF)
    wk_sb = consts.tile([128, FCH, HD], BF)
    wv_sb = consts.tile([128, FCH, HD], BF)
    wo_sb = consts.tile([128, FCH, DQ], BF)
    xt_sb = []
    for c in range(FCH):
        t = big.tile([128, S], BF, name=f"xt{c}", tag="big")
        xt_sb.append(t)
    # interleave: the first q matmul needs xt[0] + wq only
    nc.sync.dma_start(out=xt_sb[0], in_=xT[0:128, :])
    nc.sync.dma_start(out=wq_sb, in_=wq.rearrange("(c p) d -> p c d", p=128))
    nc.sync.dma_start(out=xt_sb[1], in_=xT[128:256, :])
    nc.sync.dma_start(out=wk_sb, in_=wk.rearrange("(c p) d -> p c d", p=128))
    nc.sync.dma_start(out=wv_sb, in_=wv.rearrange("(c p) d -> p c d", p=128))
    for c in range(2, FCH):
        nc.sync.dma_start(out=xt_sb[c], in_=xT[c * 128:(c + 1) * 128, :])

    # c128/s128pm: [cos;cos] and [-sin;+sin] replicated to all 4 head rows
    c128 = consts.tile([128, S], DT)
    s128 = consts.tile([128, S], DT)
    nc.sync.dma_start(out=c128[0:HD, :], in_=c64)
    nc.sync.dma_start(out=c128[HD:128, :], in_=c64)
    nc.sync.dma_start(out=s128[0:HD, :], in_=s64pm)
    nc.sync.dma_start(out=s128[HD:128, :], in_=s64pm)
    nc.sync.dma_start(out=wo_sb,
                      in_=wo.rearrange("(c p) d -> p c d", p=128))

    ident = consts.tile([128, 128], DT)
    make_identity(nc, ident)
    # bf16 identity + additive -1e9 upper-triangle: the causal mask is
    # accumulated into the diagonal score blocks by one extra PE matmul
    # (ident^T @ mneg = mneg), keeping the kb chain PE->ACT->PE only.
    ident_bf = consts.tile([128, 128], BF)
    nc.gpsimd.memset(ident_bf, 0.0)
    nc.gpsimd.affine_select(
        out=ident_bf, in_=ident_bf, compare_op=mybir.AluOpType.not_equal,
        fill=1.0, base=0, pattern=[[-1, 128]], channel_multiplier=1)
    mneg = consts.tile([128, 128], BF)
    nc.gpsimd.memset(mneg, 0.0)
    nc.gpsimd.affine_select(
        out=mneg, in_=mneg, compare_op=mybir.AluOpType.is_ge,
        fill=-1e9, base=0, pattern=[[1, 128]], channel_multiplier=-1)

    # Persistent transposed activations.
    qT_sb = qkv.tile([128, 2, S], BF)       # 2 packs x (2 heads x 64)
    # k master in fp32 (feeds the k_out output); bf16 copy duplicated in
    # both partition halves so scores lhsT can match the base partition
    # (0 or 64) of each q head's rhs slice.
    kT_f32 = qkv.tile([HD, S], DT)
    kT_bf = qkv.tile([128, S], BF)
    vT_sb = qkv.tile([HD, S], DT)           # pre-transpose v (fp32 master)
    v_ext = qkv.tile([128, NSB, 65], DT)    # v natural + ones column
    v_ext_bf = qkv.tile([128, NSB, 65], BF)
    att0 = qkv.tile([128, S], BF)           # attn out^T, heads 0,1
    att1 = qkv.tile([128, S], BF)           # attn out^T, heads 2,3

    # ---- phase 1: QKV projections + RoPE (own PSUM pool scope) ------------
    # RoPE via a second "swapped" projection computed on the PE:
    # rot(q) = q * [cos;cos] + q_swap * [-sin;+sin], all full-width DVE ops.
    with tc.tile_pool(name="psA", bufs=4, space="PSUM") as psA:
        for sp in range(NPAN):
            for pk in range(2):
                sl = slice(sp * PAN, (sp + 1) * PAN)
                q_ps = psA.tile([128, PAN], DT, name="q_ps", tag="ps")
                for c in range(FCH):
                    nc.tensor.matmul(
                        q_ps,
                        wq_sb[:, c, pk * 128:(pk + 1) * 128],
                        xt_sb[c][:, sl],
                        start=(c == 0), stop=(c == FCH - 1))
                t1 = tmp.tile([128, PAN], DT, name="rt1", tag="ropet")
                t2 = tmp.tile([128, PAN], DT, name="rt2", tag="ropet")
                nc.vector.tensor_mul(t1, q_ps, c128[:, sl])
                # swapped-half reads straight from PSUM (mixed PSUM+SBUF
                # operands may have different base partitions)
                for q in range(4):
                    lo_d, hi_d = q * 32, q * 32 + 32
                    sw = (q ^ 1) * 32
                    nc.vector.tensor_mul(t2[lo_d:hi_d, :],
                                         q_ps[sw:sw + 32, :],
                                         s128[lo_d:hi_d, sl])
                nc.vector.tensor_add(qT_sb[:, pk, sl], t1, t2)

        for sp in range(NPAN):
            sl = slice(sp * PAN, (sp + 1) * PAN)
            k_ps = psA.tile([HD, PAN], DT, name="k_ps", tag="ps")
            for c in range(FCH):
                nc.tensor.matmul(
                    k_ps, wk_sb[:, c, :], xt_sb[c][:, sl],
                    start=(c == 0), stop=(c == FCH - 1))
            t1k = tmp.tile([HD, PAN], DT, name="rt1k", tag="ropet")
            t2k = tmp.tile([HD, PAN], DT, name="rt2k", tag="ropet")
            nc.vector.tensor_mul(t1k, k_ps, c128[0:HD, sl])
            for q in range(2):
                lo_d, hi_d = q * 32, q * 32 + 32
                sw = (q ^ 1) * 32
                nc.vector.tensor_mul(t2k[lo_d:hi_d, :], k_ps[sw:sw + 32, :],
                                     s128[lo_d:hi_d, sl])
            nc.vector.tensor_add(kT_f32[:, sl], t1k, t2k)
            nc.vector.tensor_copy(kT_bf[0:HD, sl], kT_f32[:, sl])
            nc.sync.dma_start(out=kT_bf[HD:128, sl], in_=kT_bf[0:HD, sl])

        for sp in range(NPAN):
            v_ps = psA.tile([HD, PAN], DT, name="v_ps", tag="ps")
            for c in range(FCH):
                nc.tensor.matmul(
                    v_ps, wv_sb[:, c, :],
                    xt_sb[c][:, sp * PAN:(sp + 1) * PAN],
                    start=(c == 0), stop=(c == FCH - 1))
            nc.scalar.copy(vT_sb[:, sp * PAN:(sp + 1) * PAN], v_ps)

        # ---- phase 2: k/v back to natural layout for outputs + AV --------
        nc.vector.memset(v_ext[:, :, 64:65], 1.0)
        for kb in range(NSB):
            sl = slice(kb * 128, (kb + 1) * 128)
            vt_ps = psA.tile([128, HD], DT, name="vt_ps", tag="tp")
            nc.tensor.transpose(vt_ps, vT_sb[:, sl], ident[0:HD, 0:HD])
            nc.scalar.copy(v_ext[:, kb, 0:HD], vt_ps)
            nc.sync.dma_start(out=v_out[sl, :], in_=v_ext[:, kb, 0:HD])

            kt_ps = psA.tile([128, HD], DT, name="kt_ps", tag="tp")
            nc.tensor.transpose(kt_ps, kT_f32[:, sl], ident[0:HD, 0:HD])
            kn_sb = outp.tile([128, HD], DT, name="kn_sb", tag="kn")
            nc.scalar.copy(kn_sb, kt_ps)
            nc.sync.dma_start(out=k_out[sl, :], in_=kn_sb)
            nc.scalar.copy(v_ext_bf[:, kb, :], v_ext[:, kb, :])

    # ---- phase 3: causal attention, scores transposed --------------------
    # Panel-outer: both head-pairs of a 512-column panel finish together,
    # so the attn^T AllGather + o_proj for the first S/2 columns overlap
    # the attention compute of the second S/2.
    cc_ins = [dram.tile([2, 128, PAN], BF, name=f"cc_in{i}")
              for i in range(NPAN)]
    cc_outs = [dram.tile([G, 2, 128, PAN], BF, name=f"cc_out{i}")
               for i in range(NPAN)]
    at_sb = [[None] * FCH for _ in range(NPAN)]

    with tc.tile_pool(name="psS", bufs=2, space="PSUM") as psS, \
         tc.tile_pool(name="psAV", bufs=1, space="PSUM") as psAV, \
         tc.tile_pool(name="psO", bufs=2, space="PSUM") as psO:

        def o_proj_quarter(p):
            # AllGather panel p of attn^T, then its 4 o_proj s-blocks.
            for att, i in ((att0, 0), (att1, 1)):
                nc.sync.dma_start(out=cc_ins[p][i],
                                  in_=att[:, p * PAN:(p + 1) * PAN])
            nc.gpsimd.collective_compute(
                "AllGather", mybir.AluOpType.bypass,
                replica_groups=[[0, 1, 2, 3], [4, 5, 6, 7]],
                ins=[cc_ins[p].opt()], outs=[cc_outs[p].opt()])
            # cc_outs[p][r][i] holds heads {4r+2i, 4r+2i+1} = wo chunk
            # 2r+i for columns [p*PAN, (p+1)*PAN).
            for i in range(2):
                for r in range(G):
                    t = big.tile([128, PAN], BF, name=f"at{p}_{i}_{r}",
                                 tag="at")
                    nc.sync.dma_start(out=t, in_=cc_outs[p][r, i])
                    at_sb[p][2 * r + i] = t
            for sb_i in range(4):
                sl = slice(p * PAN + sb_i * 128, p * PAN + (sb_i + 1) * 128)
                o_ps = psO.tile([128, DQ], DT, name="o_ps", tag="o")
                for c in range(FCH):
                    nc.tensor.matmul(
                        o_ps, at_sb[p][c][:, sb_i * 128:(sb_i + 1) * 128],
                        wo_sb[:, c, :], start=(c == 0), stop=(c == FCH - 1))
                o_sb = outp.tile([128, DQ], DT, name="o_sb", tag="o_sb")
                nc.vector.tensor_copy(o_sb, o_ps)
                nc.sync.dma_start(out=out_s[sl, :], in_=o_sb)

        for p in range(NPAN):
            nkb = 4 * (p + 1)
            q_sl = slice(p * PAN, (p + 1) * PAN)
            for hp in range(2):
                att = att0 if hp == 0 else att1
                av_ps = psAV.tile([65, 2, PAN], DT, name="av_ps", tag="av")
                for kb in range(nkb):
                    k_sl = slice(kb * 128, (kb + 1) * 128)
                    off = (kb - 4 * p) * 128  # >=0 only on diagonal blocks
                    lo = max(off, 0)  # first valid q column in this panel
                    s_ps = psS.tile([128, 2, PAN], DT, name="s_ps", tag="s")
                    diag = off >= 0
                    for hi in range(2):
                        base = hi * 64
                        nc.tensor.matmul(
                            s_ps[:, hi, lo:],
                            kT_bf[base:base + 64, k_sl],
                            qT_sb[base:base + 64, hp,
                                  p * PAN + lo:(p + 1) * PAN],
                            start=True, stop=not diag)
                    if diag:
                        for hi in range(2):
                            nc.tensor.matmul(
                                s_ps[:, hi, off:off + 128],
                                ident_bf, mneg, start=False, stop=True)
                    pt = pt_pool.tile([128, 2, PAN], BF, name="pt", tag="pt")
                    nc.scalar.activation(pt[:, :, lo:], s_ps[:, :, lo:],
                                         EXP, scale=SCALE)
                    for hi in range(2):
                        nc.tensor.matmul(
                            av_ps[:, hi, lo:], v_ext_bf[:, kb, :],
                            pt[:, hi, lo:],
                            start=(kb == 0), stop=(kb == nkb - 1))
                # Evacuate the av bank quickly (copy unnormalized), then
                # normalize in place once 1/Z arrives via the DRAM-bounce
                # partition broadcast (engines can't partition-broadcast;
                # gpsimd must stay free for the collectives).
                z_sb = tmp.tile([1, 2, PAN], DT, name="z_sb", tag="z")
                nc.vector.tensor_copy(z_sb, av_ps[64:65, :, :])
                for hi in range(2):
                    nc.vector.tensor_copy(att[hi * 64:hi * 64 + 64, q_sl],
                                          av_ps[0:HD, hi, :])
                r_sb = tmp.tile([1, 2, PAN], DT, name="r_sb", tag="r")
                nc.vector.reciprocal_approx_fast(out=r_sb, in_=z_sb)
                r_dr = dram.tile([1, 2, PAN], DT, name="r_dr", tag="r_dr",
                                 bufs=2)
                nc.sync.dma_start(out=r_dr, in_=r_sb)
                rb = tmp.tile([128, PAN], DT, name="rb", tag="rb")
                for hi in range(2):
                    nc.sync.dma_start(
                        out=rb[hi * 64:hi * 64 + 64, :],
                        in_=r_dr[0:1, hi, :].to_broadcast([HD, PAN]))
                for hi in range(2):
                    sl_a = slice(hi * 64, hi * 64 + 64)
                    nc.vector.tensor_mul(att[sl_a, q_sl], att[sl_a, q_sl],
                                         rb[sl_a, :])
            o_proj_quarter(p)


def get_nc():
    if "nc" not in _NC_CACHE:
        _NC_CACHE["nc"] = _build_nc()
    return _NC_CACHE["nc"]


def _swap_halves(w):
    """Swap the two 32-wide halves of every 64-wide head block of w's
    columns (so q_swap = x @ w_swap has rotate-half partner rows)."""
    w = w.reshape(w.shape[0], -1, 2, HD // 2)
    return np.ascontiguousarray(w[:, :, ::-1, :].reshape(w.shape[0], -1))


def make_in_maps(x, cos, sin, wq, wk, wv, wo):
    cosT = np.asarray(cos, F32).T
    sinT = np.asarray(sin, F32).T
    c64 = np.ascontiguousarray(np.vstack([cosT, cosT]))
    s64pm = np.ascontiguousarray(np.vstack([-sinT, sinT]))
    x = np.asarray(x, F32).astype(NPBF)
    wq, wk, wv, wo = (np.asarray(a, F32).astype(NPBF)
                      for a in (wq, wk, wv, wo))
    in_maps = []
    for core in range(NCORES):
        b, g = divmod(core, G)
        in_maps.append({
            "xT": np.ascontiguousarray(x[b].T),
            "c64": c64,
            "s64pm": s64pm,
            "wq": np.ascontiguousarray(wq[:, g * DQ:(g + 1) * DQ]),
            "wk": np.ascontiguousarray(wk[:, g * HD:(g + 1) * HD]),
            "wv": np.ascontiguousarray(wv[:, g * HD:(g + 1) * HD]),
            "wo": np.ascontiguousarray(wo[:, g * DQ:(g + 1) * DQ]),
        })
    return in_maps


def assemble(results):
    out = np.empty((B, S, D), F32)
    new_k = np.empty((B, S, KVH, HD), F32)
    new_v = np.empty((B, S, KVH, HD), F32)
    for core in range(NCORES):
        b, g = divmod(core, G)
        r = results[core]
        out[b, :, g * DQ:(g + 1) * DQ] = r["out_s"]
        new_k[b, :, g, :] = r["k_out"]
        new_v[b, :, g, :] = r["v_out"]
    return out, new_k, new_v


def _ensure_ntff_hook():
    """Register the axon NTFF profile hook if the container's antenv stub
    lacks it (needed only for trace=True timing runs)."""
    import sys
    import types
    try:
        from antenv.axon_hooks import get_axon_ntff_profile_hook  # noqa: F401
        return
    except ImportError:
        pass
    try:
        import antenv
        from trn_agent_boot.trn_boot import _ntff_profile_via_ctypes
        mod = types.ModuleType("antenv.axon_hooks")
        state = {"fn": None}
        mod.set_axon_ntff_profile_hook = lambda fn: state.update(fn=fn)
        mod.get_axon_ntff_profile_hook = lambda: state["fn"]
        sys.modules["antenv.axon_hooks"] = mod
        antenv.axon_hooks = mod
        hook = _ntff_profile_via_ctypes("/opt/axon/libaxon_pjrt.so")
        if hook is not None:
            mod.set_axon_ntff_profile_hook(hook)
    except Exception as e:  # profiling is best-effort; never break the run
        print(f"ntff hook setup failed: {e}")


def kernel(x, cos, sin, mask, wq, wk, wv, wo):
    # mask is not shipped to the device: the kernel applies causality
    # structurally, which matches the reference's -1e9 additive mask.
    nc = get_nc()
    in_maps = make_in_maps(x, cos, sin, wq, wk, wv, wo)
    trace = bool(int(os.environ.get("KERNEL_TRACE", "0")))
    if trace:
        _ensure_ntff_hook()
    res = run_bass_kernel_spmd(nc, in_maps, list(range(NCORES)), trace=trace)
    if trace:
        _NC_CACHE["last_exec_time_ns"] = res.exec_time_ns
    return assemble(res.results)


# revision 25
# speedup vs baseline: 1.0211x; 1.0110x over previous
"""Trainium2 Bass kernel for nn_Attention_47725676593424.

GQA attention layer: B=2, S=2048, D=1024, H=16 q-heads, KVH=4 kv-heads,
HD=64, RoPE, causal mask, returns (out, new_k, new_v).

Sharding (8 cores): core = b*4 + g, b = batch (data parallel), g = head
group (tensor parallel). Each core computes q-heads [4g, 4g+4) and kv
head g for batch b (whole KV group local, GQA repeat is implicit), then
AllGathers the per-head attention output within its 4-core batch group
and computes a 256-column slice of the o_proj output.

Layout: activations flow transposed (feature on partitions, sequence on
the free axis) so every matmul contracts on the partition dim with zero
on-device transposes of x. Scores are computed transposed S^T[k, q]; the
softmax denominator comes free from a ones-column appended to V. The
causal mask is applied structurally (upper-triangular key blocks are
skipped / zeroed), which matches the reference's additive -1e9 mask
exactly because exp underflows to 0. Softmax runs without max
subtraction: scores*scale is bounded (|s| < ~4) for any plausible
activation scale here, so exp cannot overflow.
"""

import os
import numpy as np
from contextlib import ExitStack

import concourse.bass as bass
import concourse.mybir as mybir
import concourse.tile as tile
from concourse import bacc
from concourse.bass_utils import run_bass_kernel_spmd
from concourse.masks import make_identity

# Problem constants (hardcoded per harness contract).
B, S, D = 2, 2048, 1024
H, KVH, HD = 16, 4, 64
NCORES = 8
G = 4                 # head groups (tensor-parallel degree per batch)
HPG = H // G          # 4 q heads per core
DQ = HPG * HD         # 256 = per-core q/attn feature dim
SCALE = 1.0 / 8.0     # 1/sqrt(HD)
PAN = 512             # q panel width (one PSUM bank of fp32)
NPAN = S // PAN       # 4
SB = 128              # s block
NSB = S // SB         # 16
FCH = D // 128        # 8 feature chunks of the contraction dim
DT = mybir.dt.float32
BF = mybir.dt.bfloat16
F32 = np.float32
try:
    import ml_dtypes
    NPBF = ml_dtypes.bfloat16
except ImportError:  # pragma: no cover
    NPBF = None

_NC_CACHE = {}


def _build_nc():
    nc = bacc.Bacc("TRN2", target_bir_lowering=False, debug=False,
                   num_devices=NCORES)

    xT_h = nc.dram_tensor("xT", [D, S], BF, kind="ExternalInput")
    c64_h = nc.dram_tensor("c64", [HD, S], DT, kind="ExternalInput")
    s64_h = nc.dram_tensor("s64pm", [HD, S], DT, kind="ExternalInput")
    wq_h = nc.dram_tensor("wq", [D, DQ], BF, kind="ExternalInput")
    wk_h = nc.dram_tensor("wk", [D, HD], BF, kind="ExternalInput")
    wv_h = nc.dram_tensor("wv", [D, HD], BF, kind="ExternalInput")
    wo_h = nc.dram_tensor("wo", [D, DQ], BF, kind="ExternalInput")
    out_h = nc.dram_tensor("out_s", [S, DQ], DT, kind="ExternalOutput")
    kout_h = nc.dram_tensor("k_out", [S, HD], DT, kind="ExternalOutput")
    vout_h = nc.dram_tensor("v_out", [S, HD], DT, kind="ExternalOutput")

    xT, c64, s64pm = xT_h.ap(), c64_h.ap(), s64_h.ap()
    wq, wk, wv, wo = wq_h.ap(), wk_h.ap(), wv_h.ap(), wo_h.ap()
    out_s, k_out, v_out = out_h.ap(), kout_h.ap(), vout_h.ap()

    with ExitStack() as ctx:
        tc = ctx.enter_context(tile.TileContext(nc))
        _emit(ctx, tc, nc, xT, c64, s64pm, wq, wk, wv, wo,
              out_s, k_out, v_out)

    nc.compile()
    return nc


def _emit(ctx, tc, nc, xT, c64, s64pm, wq, wk, wv, wo,
          out_s, k_out, v_out):
    EXP = mybir.ActivationFunctionType.Exp

    consts = ctx.enter_context(tc.tile_pool(name="consts", bufs=1))
    big = ctx.enter_context(tc.tile_pool(name="big", bufs=8))
    qkv = ctx.enter_context(tc.tile_pool(name="qkv", bufs=1))
    pt_pool = ctx.enter_context(tc.tile_pool(name="pt", bufs=3))
    tmp = ctx.enter_context(tc.tile_pool(name="tmp", bufs=6))
    outp = ctx.enter_context(tc.tile_pool(name="outp", bufs=3))
    dram = ctx.enter_context(tc.tile_pool(name="dram", bufs=1, space="DRAM"))

    # ---- inputs into SBUF (x and projection weights first: they gate the
    # first matmuls; rope tables and wo aren't needed until later) --------
    wq_sb = consts.tile([128, FCH, DQ], BF)
    wk_sb = consts.tile([128, FCH, HD], BF)
    wv_sb = consts.tile([128, FCH, HD], BF)
    wo_sb = consts.tile([128, FCH, DQ], BF)
    xt_sb = []
    for c in range(FCH):
        t = big.tile([128, S], BF, name=f"xt{c}", tag="big")
        xt_sb.append(t)
    # interleave: the first q matmul needs xt[0] + wq only
    nc.sync.dma_start(out=xt_sb[0], in_=xT[0:128, :])
    nc.sync.dma_start(out=wq_sb, in_=wq.rearrange("(c p) d -> p c d", p=128))
    nc.sync.dma_start(out=xt_sb[1], in_=xT[128:256, :])
    nc.sync.dma_start(out=wk_sb, in_=wk.rearrange("(c p) d -> p c d", p=128))
    nc.sync.dma_start(out=wv_sb, in_=wv.rearrange("(c p) d -> p c d", p=128))
    for c in range(2, FCH):
        nc.sync.dma_start(out=xt_sb[c], in_=xT[c * 128:(c + 1) * 128, :])

    # c128/s128pm: [cos;cos] and [-sin;+sin] replicated to all 4 head rows
    c128 = consts.tile([128, S], DT)
    s128 = consts.tile([128, S], DT)
    nc.sync.dma_start(out=c128[0:HD, :], in_=c64)
    nc.sync.dma_start(out=c128[HD:128, :], in_=c64)
    nc.sync.dma_start(out=s128[0:HD, :], in_=s64pm)
    nc.sync.dma_start(out=s128[HD:128, :], in_=s64pm)
    nc.sync.dma_start(out=wo_sb,
                      in_=wo.rearrange("(c p) d -> p c d", p=128))

    ident = consts.tile([128, 128], DT)
    make_identity(nc, ident)
    # bf16 identity + additive -1e9 upper-triangle: the causal mask is
    # accumulated into the diagonal score blocks by one extra PE matmul
    # (ident^T @ mneg = mneg), keeping the kb chain PE->ACT->PE only.
    ident_bf = consts.tile([128, 128], BF)
    nc.gpsimd.memset(ident_bf, 0.0)
    nc.gpsimd.affine_select(
        out=ident_bf, in_=ident_bf, compare_op=mybir.AluOpType.not_equal,
        fill=1.0, base=0, pattern=[[-1, 128]], channel_multiplier=1)
    mneg = consts.tile([128, 128], BF)
    nc.gpsimd.memset(mneg, 0.0)
    nc.gpsimd.affine_select(
        out=mneg, in_=mneg, compare_op=mybir.AluOpType.is_ge,
        fill=-1e9, base=0, pattern=[[1, 128]], channel_multiplier=-1)

    # Persistent transposed activations.
    qT_sb = qkv.tile([128, 2, S], BF)       # 2 packs x (2 heads x 64)
    # k master in fp32 (feeds the k_out output); bf16 copy duplicated in
    # both partition halves so scores lhsT can match the base partition
    # (0 or 64) of each q head's rhs slice.
    kT_f32 = qkv.tile([HD, S], DT)
    kT_bf = qkv.tile([128, S], BF)
    vT_sb = qkv.tile([HD, S], DT)           # pre-transpose v (fp32 master)
    v_ext = qkv.tile([128, NSB, 65], DT)    # v natural + ones column
    v_ext_bf = qkv.tile([128, NSB, 65], BF)
    att0 = qkv.tile([128, S], BF)           # attn out^T, heads 0,1
    att1 = qkv.tile([128, S], BF)           # attn out^T, heads 2,3

    # ---- phase 1: QKV projections + RoPE (own PSUM pool scope) ------------
    # rot(q) = q * [cos;cos] + swap(q) * [-sin;+sin]; the swapped-half
    # reads come straight from the projection PSUM (mixed PSUM+SBUF
    # operands may use different base partitions).
    with tc.tile_pool(name="psA", bufs=4, space="PSUM") as psA:
        for sp in range(NPAN):
            for pk in range(2):
                sl = slice(sp * PAN, (sp + 1) * PAN)
                q_ps = psA.tile([128, PAN], DT, name="q_ps", tag="ps")
                for c in range(FCH):
                    nc.tensor.matmul(
                        q_ps,
                        wq_sb[:, c, pk * 128:(pk + 1) * 128],
                        xt_sb[c][:, sl],
                        start=(c == 0), stop=(c == FCH - 1))
                t1 = tmp.tile([128, PAN], DT, name="rt1", tag="ropet")
                t2 = tmp.tile([128, PAN], DT, name="rt2", tag="ropet")
                nc.vector.tensor_mul(t1, q_ps, c128[:, sl])
                # swapped-half reads straight from PSUM (mixed PSUM+SBUF
                # operands may have different base partitions)
                for q in range(4):
                    lo_d, hi_d = q * 32, q * 32 + 32
                    sw = (q ^ 1) * 32
                    nc.vector.tensor_mul(t2[lo_d:hi_d, :],
                                         q_ps[sw:sw + 32, :],
                                         s128[lo_d:hi_d, sl])
                nc.vector.tensor_add(qT_sb[:, pk, sl], t1, t2)

        for sp in range(NPAN):
            sl = slice(sp * PAN, (sp + 1) * PAN)
            k_ps = psA.tile([HD, PAN], DT, name="k_ps", tag="ps")
            for c in range(FCH):
                nc.tensor.matmul(
                    k_ps, wk_sb[:, c, :], xt_sb[c][:, sl],
                    start=(c == 0), stop=(c == FCH - 1))
            t1k = tmp.tile([HD, PAN], DT, name="rt1k", tag="ropet")
            t2k = tmp.tile([HD, PAN], DT, name="rt2k", tag="ropet")
            nc.vector.tensor_mul(t1k, k_ps, c128[0:HD, sl])
            for q in range(2):
                lo_d, hi_d = q * 32, q * 32 + 32
                sw = (q ^ 1) * 32
                nc.vector.tensor_mul(t2k[lo_d:hi_d, :], k_ps[sw:sw + 32, :],
                                     s128[lo_d:hi_d, sl])
            nc.vector.tensor_add(kT_f32[:, sl], t1k, t2k)
            nc.vector.tensor_copy(kT_bf[0:HD, sl], kT_f32[:, sl])
            nc.sync.dma_start(out=kT_bf[HD:128, sl], in_=kT_bf[0:HD, sl])

        for sp in range(NPAN):
            v_ps = psA.tile([HD, PAN], DT, name="v_ps", tag="ps")
            for c in range(FCH):
                nc.tensor.matmul(
                    v_ps, wv_sb[:, c, :],
                    xt_sb[c][:, sp * PAN:(sp + 1) * PAN],
                    start=(c == 0), stop=(c == FCH - 1))
            nc.scalar.copy(vT_sb[:, sp * PAN:(sp + 1) * PAN], v_ps)

        # ---- phase 2: k/v back to natural layout for outputs + AV --------
        nc.vector.memset(v_ext[:, :, 64:65], 1.0)
        for kb in range(NSB):
            sl = slice(kb * 128, (kb + 1) * 128)
            vt_ps = psA.tile([128, HD], DT, name="vt_ps", tag="tp")
            nc.tensor.transpose(vt_ps, vT_sb[:, sl], ident[0:HD, 0:HD])
            nc.scalar.copy(v_ext[:, kb, 0:HD], vt_ps)
            nc.sync.dma_start(out=v_out[sl, :], in_=v_ext[:, kb, 0:HD])

            kt_ps = psA.tile([128, HD], DT, name="kt_ps", tag="tp")
            nc.tensor.transpose(kt_ps, kT_f32[:, sl], ident[0:HD, 0:HD])
            kn_sb = outp.tile([128, HD], DT, name="kn_sb", tag="kn")
            nc.scalar.copy(kn_sb, kt_ps)
            nc.sync.dma_start(out=k_out[sl, :], in_=kn_sb)
            nc.scalar.copy(v_ext_bf[:, kb, :], v_ext[:, kb, :])

    # ---- phase 3: causal attention, scores transposed --------------------
    # Panel-outer: both head-pairs of a 512-column panel finish together,
    # so the attn^T AllGather + o_proj for the first S/2 columns overlap
    # the attention compute of the second S/2.
    cc_ins = [dram.tile([2, 128, PAN], BF, name=f"cc_in{i}")
              for i in range(NPAN)]
    cc_outs = [dram.tile([G, 2, 128, PAN], BF, name=f"cc_out{i}")
               for i in range(NPAN)]
    at_sb = [[None] * FCH for _ in range(NPAN)]

    with tc.tile_pool(name="psS", bufs=2, space="PSUM") as psS, \
         tc.tile_pool(name="psAV", bufs=1, space="PSUM") as psAV, \
         tc.tile_pool(name="psO", bufs=2, space="PSUM") as psO:

        def o_proj_quarter(p):
            # AllGather panel p of attn^T, then its 4 o_proj s-blocks.
            for att, i in ((att0, 0), (att1, 1)):
                nc.sync.dma_start(out=cc_ins[p][i],
                                  in_=att[:, p * PAN:(p + 1) * PAN])
            nc.gpsimd.collective_compute(
                "AllGather", mybir.AluOpType.bypass,
                replica_groups=[[0, 1, 2, 3], [4, 5, 6, 7]],
                ins=[cc_ins[p].opt()], outs=[cc_outs[p].opt()])
            # cc_outs[p][r][i] holds heads {4r+2i, 4r+2i+1} = wo chunk
            # 2r+i for columns [p*PAN, (p+1)*PAN).
            for i in range(2):
                for r in range(G):
                    t = big.tile([128, PAN], BF, name=f"at{p}_{i}_{r}",
                                 tag="at")
                    nc.sync.dma_start(out=t, in_=cc_outs[p][r, i])
                    at_sb[p][2 * r + i] = t
            for sb_i in range(4):
                sl = slice(p * PAN + sb_i * 128, p * PAN + (sb_i + 1) * 128)
                o_ps = psO.tile([128, DQ], DT, name="o_ps", tag="o")
                for c in range(FCH):
                    nc.tensor.matmul(
                        o_ps, at_sb[p][c][:, sb_i * 128:(sb_i + 1) * 128],
                        wo_sb[:, c, :], start=(c == 0), stop=(c == FCH - 1))
                o_sb = outp.tile([128, DQ], DT, name="o_sb", tag="o_sb")
                nc.vector.tensor_copy(o_sb, o_ps)
                nc.sync.dma_start(out=out_s[sl, :], in_=o_sb)

        for p in range(NPAN):
            nkb = 4 * (p + 1)
            q_sl = slice(p * PAN, (p + 1) * PAN)
            for hp in range(2):
                att = att0 if hp == 0 else att1
                av_ps = psAV.tile([65, 2, PAN], DT, name="av_ps", tag="av")
                for kb in range(nkb):
                    k_sl = slice(kb * 128, (kb + 1) * 128)
                    off = (kb - 4 * p) * 128  # >=0 only on diagonal blocks
                    lo = max(off, 0)  # first valid q column in this panel
                    s_ps = psS.tile([128, 2, PAN], DT, name="s_ps", tag="s")
                    diag = off >= 0
                    for hi in range(2):
                        base = hi * 64
                        nc.tensor.matmul(
                            s_ps[:, hi, lo:],
                            kT_bf[base:base + 64, k_sl],
                            qT_sb[base:base + 64, hp,
                                  p * PAN + lo:(p + 1) * PAN],
                            start=True, stop=not diag)
                    if diag:
                        for hi in range(2):
                            nc.tensor.matmul(
                                s_ps[:, hi, off:off + 128],
                                ident_bf, mneg, start=False, stop=True)
                    pt = pt_pool.tile([128, 2, PAN], BF, name="pt", tag="pt")
                    nc.scalar.activation(pt[:, :, lo:], s_ps[:, :, lo:],
                                         EXP, scale=SCALE)
                    for hi in range(2):
                        nc.tensor.matmul(
                            av_ps[:, hi, lo:], v_ext_bf[:, kb, :],
                            pt[:, hi, lo:],
                            start=(kb == 0), stop=(kb == nkb - 1))
                # Evacuate the av bank quickly (copy unnormalized), then
                # normalize in place once 1/Z arrives via the DRAM-bounce
                # partition broadcast (engines can't partition-broadcast;
                # gpsimd must stay free for the collectives).
                z_sb = tmp.tile([1, 2, PAN], DT, name="z_sb", tag="z")
                nc.vector.tensor_copy(z_sb, av_ps[64:65, :, :])
                for hi in range(2):
                    nc.vector.tensor_copy(att[hi * 64:hi * 64 + 64, q_sl],
                                          av_ps[0:HD, hi, :])
                r_sb = tmp.tile([1, 2, PAN], DT, name="r_sb", tag="r")
                nc.vector.reciprocal_approx_fast(out=r_sb, in_=z_sb)
                r_dr = dram.tile([1, 2, PAN], DT, name="r_dr", tag="r_dr",
                                 bufs=2)
                nc.sync.dma_start(out=r_dr, in_=r_sb)
                rb = tmp.tile([128, PAN], DT, name="rb", tag="rb")
                for hi in range(2):
                    nc.sync.dma_start(
                        out=rb[hi * 64:hi * 64 + 64, :],
                        in_=r_dr[0:1, hi, :].to_broadcast([HD, PAN]))
                for hi in range(2):
                    sl_a = slice(hi * 64, hi * 64 + 64)
                    nc.vector.tensor_mul(att[sl_a, q_sl], att[sl_a, q_sl],
                                         rb[sl_a, :])
            o_proj_quarter(p)


def get_nc():
    if "nc" not in _NC_CACHE:
        _NC_CACHE["nc"] = _build_nc()
    return _NC_CACHE["nc"]


def make_in_maps(x, cos, sin, wq, wk, wv, wo):
    cosT = np.asarray(cos, F32).T
    sinT = np.asarray(sin, F32).T
    c64 = np.ascontiguousarray(np.vstack([cosT, cosT]))
    s64pm = np.ascontiguousarray(np.vstack([-sinT, sinT]))
    x = np.asarray(x, F32).astype(NPBF)
    wq, wk, wv, wo = (np.asarray(a, F32).astype(NPBF)
                      for a in (wq, wk, wv, wo))
    in_maps = []
    for core in range(NCORES):
        b, g = divmod(core, G)
        in_maps.append({
            "xT": np.ascontiguousarray(x[b].T),
            "c64": c64,
            "s64pm": s64pm,
            "wq": np.ascontiguousarray(wq[:, g * DQ:(g + 1) * DQ]),
            "wk": np.ascontiguousarray(wk[:, g * HD:(g + 1) * HD]),
            "wv": np.ascontiguousarray(wv[:, g * HD:(g + 1) * HD]),
            "wo": np.ascontiguousarray(wo[:, g * DQ:(g + 1) * DQ]),
        })
    return in_maps


def assemble(results):
    out = np.empty((B, S, D), F32)
    new_k = np.empty((B, S, KVH, HD), F32)
    new_v = np.empty((B, S, KVH, HD), F32)
    for core in range(NCORES):
        b, g = divmod(core, G)
        r = results[core]
        out[b, :, g * DQ:(g + 1) * DQ] = r["out_s"]
        new_k[b, :, g, :] = r["k_out"]
        new_v[b, :, g, :] = r["v_out"]
    return out, new_k, new_v


def _ensure_ntff_hook():
    """Register the axon NTFF profile hook if the container's antenv stub
    lacks it (needed only for trace=True timing runs)."""
    import sys
    import types
    try:
        from antenv.axon_hooks import get_axon_ntff_profile_hook  # noqa: F401
        return
    except ImportError:
        pass
    try:
        import antenv
        from trn_agent_boot.trn_boot import _ntff_profile_via_ctypes
        mod = types.ModuleType("antenv.axon_hooks")
        state = {"fn": None}
        mod.set_axon_ntff_profile_hook = lambda fn: state.update(fn=fn)
        mod.get_axon_ntff_profile_hook = lambda: state["fn"]
        sys.modules["antenv.axon_hooks"] = mod
        antenv.axon_hooks = mod
        hook = _ntff_profile_via_ctypes("/opt/axon/libaxon_pjrt.so")
        if hook is not None:
            mod.set_axon_ntff_profile_hook(hook)
    except Exception as e:  # profiling is best-effort; never break the run
        print(f"ntff hook setup failed: {e}")


def kernel(x, cos, sin, mask, wq, wk, wv, wo):
    # mask is not shipped to the device: the kernel applies causality
    # structurally, which matches the reference's -1e9 additive mask.
    nc = get_nc()
    in_maps = make_in_maps(x, cos, sin, wq, wk, wv, wo)
    trace = bool(int(os.environ.get("KERNEL_TRACE", "0")))
    if trace:
        _ensure_ntff_hook()
    res = run_bass_kernel_spmd(nc, in_maps, list(range(NCORES)), trace=trace)
    if trace:
        _NC_CACHE["last_exec_time_ns"] = res.exec_time_ns
    return assemble(res.results)
